# revision 1
# baseline (speedup 1.0000x reference)
"""Betti-matching-loss preprocessing kernel for 8 TRN2 NeuronCores.

Reference computation (per full input of shape (B=4, C=1, D=128, H=256, W=256)):
    pred_super   = 1 - maxpool3d_2x(sigmoid(input))   -> sigmoid is monotone, so
                 = sigmoid(-maxpool3d_2x(input))
    target_super = 1 - (maxpool3d_2x(target) > 0.5)   = (maxpool3d_2x(target) <= 0.5)
    out = stack([pred_super, target_super])           # (2, B, C, 64, 128, 128)

Sharding: pure data parallel. 8 shards = 4 batch samples x 2 D-halves of 64
planes each (the D split at an even index never crosses a pool window).

Per-core kernel: load 8 contiguous (256,256) planes per 2 MB DMA into a
[128, 4096] SBUF tile laid out so partition p holds rows (2p, 2p+1) of each
plane; then a 3-level pairwise tensor_max tree pools D, H, W; then the
pointwise op (sigmoid on the scalar engine / <=0.5 compare on DVE).  Stores
are deferred to a final burst so the steady-state DMA window is pure HBM
reads (~349 GB/s, at the per-core HBM limit); mixed read/write interleaving
measured ~20 us slower.
"""

import numpy as np

import bass_rust
import concourse.bass as bass
import concourse.mybir as mybir
import concourse.tile as tile
from concourse.bass_utils import run_bass_kernel_spmd
from concourse.vector_clock import ScopedClock

f32 = mybir.dt.float32


def _patched_drain_and_barrier(self, tick_clock, wait_clock):
    """Replacement for TileContext._drain_and_barrier.

    The stock version hangs every outstanding semaphore wait on one Drain
    instruction; the walrus in this environment rejects >1 sync-wait per
    non-EventSemaphore instruction ("Too many sync wait commands").  Emit
    one sequencer NOP per semaphore wait instead, then drain + barrier.
    """
    ((_, vclock),) = ScopedClock({None: tick_clock.global_clock}).items()
    ticks = list(vclock)
    for proc_idx, sem in self.sems.allocated().items():
        t = ticks[proc_idx]
        if t > 0:
            self.nc.sync.nop()._wait_ge(sem, bass_rust.tick_to_sem(t, proc_idx))
    self.nc.sync.drain()
    self.nc.all_engine_barrier(sem_only=True)
    popped = self.nc._tile_sem_poison_stack.pop()
    assert popped is self._sem_poison
    self.nc.clear_and_free_semaphores(list(self.sems.allocated().values()))


tile.TileContext._drain_and_barrier = _patched_drain_and_barrier


def _split_excess_waits(nc: bass.Bass) -> None:
    """Walrus in this env caps sync-waits at 1 per instruction (2 for
    EventSemaphore).  Move excess waits onto same-engine NoOps inserted
    immediately before the offending instruction."""
    for f in nc.m.functions:
        for bb in f.blocks:
            insts = bb.instructions
            out = []
            changed = False
            for inst in insts:
                si = inst.sync_info
                cap = 2 if type(inst).__name__ == "InstEventSemaphore" else 1
                if si is not None and len(si.on_wait) > cap:
                    w = list(si.on_wait)
                    for k, extra in enumerate(w[cap:]):
                        nop = mybir.InstNoOp(
                            name=f"{inst.name}-xw{k}",
                            engine=inst.engine,
                            sync_info=mybir.SyncInfo(
                                on_wait=[extra], on_update=[]
                            ),
                            bass_nofuse=True,
                        )
                        nc.register_instruction(nop, overwrite=True)
                        out.append(nop)
                    inst.sync_info = mybir.SyncInfo(
                        on_wait=w[:cap], on_update=si.on_update
                    )
                    changed = True
                out.append(inst)
            if changed:
                bb.instructions = out

B, C, D, H, W = 4, 1, 128, 256, 256
NCORES = 8
D_SH = D // 2      # 64 input planes per core
DZ = D_SH // 2     # 32 output planes per core
HO, WO = H // 2, W // 2
PPT = 8            # input planes per load tile (2 MB DMAs)


def build_nc(d_sh: int = D_SH, ppt: int = PPT) -> bass.Bass:
    nt = d_sh // ppt       # load tiles per tensor
    zt = ppt // 2          # output planes per load tile
    dz = d_sh // 2
    nc = bass.Bass()
    inp = nc.declare_dram_parameter("input", [d_sh, H, W], f32, isOutput=False)
    tgt = nc.declare_dram_parameter("target", [d_sh, H, W], f32, isOutput=False)
    out = nc.declare_dram_parameter("out", [2, dz, HO, WO], f32, isOutput=True)

    # chunk schedule: full tiles, last full tile split in half to shorten
    # the final compute drain-down
    chunks = [(q * ppt, ppt) for q in range(nt - 1)]
    last = (nt - 1) * ppt
    if ppt >= 8:
        chunks += [(last, ppt // 2), (last + ppt // 2, ppt // 2)]
    else:
        chunks += [(last, ppt)]

    n_g = 2 * len(chunks)  # one g tile per (chunk, tensor), all kept live
    with tile.TileContext(nc) as tc:
        with (
            tc.tile_pool(name="load", bufs=7) as load_pool,
            tc.tile_pool(name="lvl1", bufs=3) as pool1,
            tc.tile_pool(name="lvl2", bufs=3) as pool2,
            tc.tile_pool(name="lvl3", bufs=3) as pool3,
            tc.tile_pool(name="post", bufs=n_g) as pool4,
        ):
            deferred_stores = []
            flush_after = max(0, len(chunks) - 3)  # mid-window flush point
            for ci, (d0, cs) in enumerate(chunks):
                zt_q = cs // 2
                for which, src in ((0, inp), (1, tgt)):
                    # ---- load cs planes; partition p <- rows (2p, 2p+1) ----
                    t = load_pool.tile([128, ppt * 512], f32, tag="load")
                    sv = src[d0:d0 + cs].rearrange(
                        "d (h2 hp) w -> h2 d hp w", hp=2
                    )
                    dv = t.rearrange("p (d hp w) -> p d hp w", d=ppt, hp=2)[
                        :, :cs
                    ]
                    nc.sync.dma_start(dv, sv)

                    # ---- level 1: pool D (pairs of planes) ----
                    # (this walrus only codegens TensorTensor on DVE)
                    u = pool1.tile([128, (ppt // 2) * 512], f32, tag="u")
                    tv = t.rearrange("p (z two blk) -> p z two blk", two=2, blk=512)
                    nc.vector.tensor_max(
                        u.rearrange("p (z blk) -> p z blk", blk=512)[:, :zt_q],
                        tv[:, :zt_q, 0, :],
                        tv[:, :zt_q, 1, :],
                    )

                    # ---- level 2: pool H (row 2p vs 2p+1, free-dim halves) ----
                    v = pool2.tile([128, (ppt // 2) * 256], f32, tag="v")
                    uv = u.rearrange("p (z hp w) -> p z hp w", hp=2, w=256)
                    nc.vector.tensor_max(
                        v.rearrange("p (z w) -> p z w", w=256)[:, :zt_q],
                        uv[:, :zt_q, 0, :],
                        uv[:, :zt_q, 1, :],
                    )

                    # ---- level 3: pool W (even/odd columns) ----
                    o = pool3.tile([128, (ppt // 2) * 128], f32, tag="o")
                    vv = v.rearrange("p (z w two) -> p z w two", w=WO, two=2)
                    nc.vector.tensor_max(
                        o.rearrange("p (z w) -> p z w", w=128)[:, :zt_q],
                        vv[:, :zt_q, :, 0],
                        vv[:, :zt_q, :, 1],
                    )

                    # ---- pointwise ----
                    g = pool4.tile([128, (ppt // 2) * 128], f32, tag="g")
                    if which == 0:
                        nc.scalar.activation(
                            g[:, :zt_q * 128], o[:, :zt_q * 128],
                            mybir.ActivationFunctionType.Sigmoid,
                            bias=0.0, scale=-1.0,
                        )
                    else:
                        nc.vector.tensor_scalar(
                            g[:, :zt_q * 128], o[:, :zt_q * 128],
                            0.5, None, mybir.AluOpType.is_le,
                        )

                    # ---- defer the store: keep the main window pure-read ----
                    dst = out[which, d0 // 2:d0 // 2 + zt_q].rearrange(
                        "z h w -> h z w"
                    )
                    gsrc = g.rearrange("p (z w) -> p z w", w=WO)[:, :zt_q]
                    deferred_stores.append((dst, gsrc))

                # flush the accumulated stores once mid-window so the final
                # burst (and the ACT-ring FIFO backlog behind the last
                # chunk's store) is short
                if ci == flush_after:
                    for dst, gsrc in deferred_stores:
                        nc.scalar.dma_start(dst, gsrc)
                    deferred_stores = []

            # remaining store burst at the end (ACT ring)
            for dst, gsrc in deferred_stores:
                nc.scalar.dma_start(dst, gsrc)
    _split_excess_waits(nc)
    return nc


_NC_CACHE: dict = {}


def kernel(input: np.ndarray, target: np.ndarray) -> np.ndarray:
    input = np.asarray(input, dtype=np.float32)
    target = np.asarray(target, dtype=np.float32)
    assert input.shape == (B, C, D, H, W), input.shape

    if "nc" not in _NC_CACHE:
        _NC_CACHE["nc"] = build_nc()
    nc = _NC_CACHE["nc"]

    in_maps = []
    for i in range(NCORES):
        b, half = divmod(i, 2)
        sl = slice(half * D_SH, (half + 1) * D_SH)
        in_maps.append({
            "input": np.ascontiguousarray(input[b, 0, sl]),
            "target": np.ascontiguousarray(target[b, 0, sl]),
        })

    res = run_bass_kernel_spmd(nc, in_maps, core_ids=list(range(NCORES))).results

    full = np.empty((2, B, C, D // 2, HO, WO), dtype=np.float32)
    for i in range(NCORES):
        b, half = divmod(i, 2)
        full[:, b, 0, half * DZ:(half + 1) * DZ] = res[i]["out"]
    return full

